# revision 24
# baseline (speedup 1.0000x reference)
"""Fused transformer block (RMSNorm + qk-norm attention + MLP) for TRN2, 8 cores.

Sharding: 8 cores = (4 batches) x (2 query-halves). Each core gets its batch's
full sequence with rows rotated so its query half is rows 0..1023 (attention is
permutation-invariant over keys, so K/V row order doesn't matter). No
collectives needed; each core produces a disjoint [1024, 768] output slice.

Key performance facts this kernel is built around:
  - The PE HAM clock gate only holds 2.4 GHz when the array footprint stays
    high: every attention matmul is framed as a full 128x128-footprint op
    (zero-padded per-head K^T for logits; 128-wide over-read V windows and a
    full-footprint selector broadcast for the softmax divide).
  - ScalarE exp over the 25M logits (~206us) is the attention-phase floor;
    everything else overlaps under it.
  - DVE is strict-FIFO: big memsets go to GpSimd, PSUM evacuations are
    emitted before slow 1-lane reciprocals, and transpose-dependent fixups
    (KTz split copies, Q^T kqsc scaling) run on GpSimd so DMA waits don't
    stall the DVE pipeline.
  - Phase A builds x_hat^T with per-tile PE transposes (no DRAM round trip);
    K/Q/x2 transposes go through DRAM in 512-row chunks (>=128-col sources
    only: narrower DMA-transposes fall back to a descriptor explosion),
    pipelined so the next phase starts before the last chunk lands.
"""

import numpy as np
from contextlib import ExitStack

import concourse.bass as bass
import concourse.tile as tile
from concourse import bacc, mybir
from concourse.bass_utils import run_bass_kernel_spmd

F32 = mybir.dt.float32
BF16 = mybir.dt.bfloat16
AF = mybir.ActivationFunctionType
OP = mybir.AluOpType

B, S, D, H, HD, MLP = 4, 2048, 768, 12, 64, 3072
SQ = S // 2            # query rows per core
NT_S = S // 128        # 16 sequence tiles
NT_Q = SQ // 128       # 8 query tiles
NT_D = D // 128        # 6 model-dim tiles
NT_M = MLP // 128      # 24 mlp-dim tiles
EPS = 1e-6
VW = HD + 1            # V width incl. ones column
CH = 512               # transpose chunk (rows)
NC_S = S // CH         # 4 chunks over full sequence
NC_Q = SQ // CH        # 2 chunks over query rows


def _chunks(n):
    out, ofs = [], 0
    while ofs < n:
        c = min(512, n - ofs)
        out.append((ofs, c))
        ofs += c
    return out


def build_nc(sim_compat=False):
    nc = bacc.Bacc("TRN2", target_bir_lowering=False, debug=False, num_devices=8)

    lat = nc.dram_tensor("lat", [S, D], F32, kind="ExternalInput").ap()
    ident = nc.dram_tensor("ident", [128, 128], BF16, kind="ExternalInput").ap()
    wq = nc.dram_tensor("wq", [D, D], BF16, kind="ExternalInput").ap()
    wk = nc.dram_tensor("wk", [D, D], BF16, kind="ExternalInput").ap()
    wv = nc.dram_tensor("wv", [D, D], BF16, kind="ExternalInput").ap()
    wo = nc.dram_tensor("wo", [D, D], BF16, kind="ExternalInput").ap()
    wi = nc.dram_tensor("wi", [D, MLP], BF16, kind="ExternalInput").ap()
    wom = nc.dram_tensor("wom", [MLP, D], BF16, kind="ExternalInput").ap()
    kqsc = nc.dram_tensor("kqsc", [128, 1], F32, kind="ExternalInput").ap()
    out = nc.dram_tensor("out", [SQ, D], F32, kind="ExternalOutput").ap()

    with tile.TileContext(nc) as tc, ExitStack() as top:
        def ptile(pool, shape, dtype, name):
            return pool.tile(shape, dtype, name=name, tag=name)

        p_const = top.enter_context(tc.tile_pool(name="p_const", bufs=1))
        p_x2 = top.enter_context(tc.tile_pool(name="p_x2", bufs=1))
        p_oT = tc.alloc_tile_pool(name="p_oT", bufs=1)
        p_att = tc.alloc_tile_pool(name="p_att", bufs=1)

        # ---- persistent tiles ----
        # Vaug layout per (t, head): [64 V cols | ones col]; attn@V reads a
        # 128-wide stationary window (full PE-array footprint keeps the HAM
        # clock at 2.4 GHz) that runs into the next head's block (harmless:
        # junk lands in PSUM rows 65-127). The softmax denominator lands in
        # PSUM row 64 so the whole [128,SQ] accumulator is evacuated and
        # re-broadcast with full-footprint aligned ops. 63-col zero tail
        # covers the last head's window.
        Vaug = ptile(p_att, [128, NT_S * H * VW + 63], BF16, name="Vaug")
        oT = ptile(p_oT, [128, NT_D * SQ], BF16, name="oT")
        kqsc_t = ptile(p_const, [128, 1], F32, name="kqsc_t")
        sel_t = ptile(p_const, [128, 128], BF16, name="sel_t")
        eps_t = ptile(p_const, [128, 1], F32, name="eps_t")
        ident_t = ptile(p_const, [128, 128], BF16, name="ident_t")
        nc.sync.dma_start(ident_t[:], ident[:])
        # Per-head K^T with the head's 64 rows at partition 64*(h%2) and
        # zeros in the other 64: logits matmuls contract over the full 128
        # partitions (zero rows contribute nothing) so the PE array runs at
        # 100% footprint instead of 50% — keeps HAM unthrottled.
        KTz = [ptile(p_att, [128, S], BF16, name=f"KTz{h}") for h in range(H)]
        QT = [ptile(p_att, [128, SQ], BF16, name=f"QT{d}") for d in range(NT_D)]
        x2 = [ptile(p_x2, [128, D], F32, name=f"x2_{q}") for q in range(NT_Q)]
        x2T = [ptile(p_x2, [128, SQ], BF16, name=f"x2T{d}") for d in range(NT_D)]

        nc.sync.dma_start(kqsc_t[:], kqsc[:])
        nc.vector.memset(eps_t[:], EPS)
        # big zero-fills go on the idle GpSimd queue: DVE is strict FIFO and
        # fills here would stall phase A's rmsnorm chain behind them
        nc.gpsimd.memset(sel_t[:], 0.0)
        nc.vector.memset(sel_t[HD:HD + 1, :], 1.0)
        nc.gpsimd.memset(Vaug[:, NT_S * H * VW:], 0.0)
        for h in range(H):
            e = h % 2
            nc.gpsimd.memset(KTz[h][64 * (1 - e):64 * (1 - e) + 64, :], 0.0)
        vview = Vaug[:, 0:NT_S * H * VW].rearrange(
            "p (s h k) -> p s h k", s=NT_S, h=H)
        nc.gpsimd.memset(vview[:, :, :, HD:VW], 1.0)

        dram = top.enter_context(tc.tile_pool(name="dram", bufs=1, space="DRAM"))
        kh_d = dram.tile([S, D], BF16, name="kh_d")
        qh_d = dram.tile([SQ, D], BF16, name="qh_d")
        x2h_d = dram.tile([SQ, D], BF16, name="x2h_d")

        # =============== Phase A: ln1 + x_hat^T ===============
        # x_hat^T is built per-tile with PE transposes (no DRAM round trip,
        # no end-of-phase barrier): projections for tile t can start as soon
        # as tile t is transposed.
        p_xT = tc.alloc_tile_pool(name="p_xT", bufs=1)
        xTa = ptile(p_xT, [128, NT_D * S], BF16, name="xTa")
        xT = [xTa[:, d * S:(d + 1) * S] for d in range(NT_D)]
        with ExitStack() as ctx:
            io = ctx.enter_context(tc.tile_pool(name="a_io", bufs=5))
            st_p = ctx.enter_context(tc.tile_pool(name="a_stats", bufs=8))
            scr = ctx.enter_context(tc.tile_pool(name="a_scr", bufs=5))
            tp_ps = ctx.enter_context(tc.tile_pool(name="a_tps", bufs=4, space="PSUM"))
            for t in range(NT_S):
                lt = io.tile([128, D], F32, name="lt")
                nc.sync.dma_start(lt[:], lat[t * 128:(t + 1) * 128, :])
                sq = scr.tile([128, D], F32, name="sq")
                ssq = st_p.tile([128, 1], F32, name="ssq")
                nc.scalar.activation(sq[:], lt[:], AF.Square, accum_out=ssq[:])
                srt = st_p.tile([128, 1], F32, name="srt")
                nc.scalar.activation(srt[:], ssq[:], AF.Sqrt, bias=eps_t[:], scale=1.0 / D)
                rs = st_p.tile([128, 1], F32, name="rs")
                nc.vector.reciprocal(rs[:], srt[:])
                xh = scr.tile([128, D], BF16, name="xh")
                nc.vector.tensor_scalar_mul(xh[:], lt[:], rs[:])
                tps = tp_ps.tile([128, NT_D * 128], BF16, name="tps")
                for d in range(NT_D):
                    nc.tensor.transpose(tps[:, d * 128:(d + 1) * 128],
                                        xh[:, d * 128:(d + 1) * 128],
                                        ident_t[:])
                nc.vector.tensor_copy(
                    xTa[:].rearrange("p (d s) -> p d s", d=NT_D)[
                        :, :, t * 128:(t + 1) * 128],
                    tps[:].rearrange("p (d s) -> p d s", d=NT_D))

        # =============== Phase B: Q/K/V projections + qk-norm ===============
        with ExitStack() as ctx:
            wp = ctx.enter_context(tc.tile_pool(name="b_w", bufs=1))
            wq_sb = [wp.tile([128, D], BF16, name=f"wq_sb{d}") for d in range(NT_D)]
            wk_sb = [wp.tile([128, D], BF16, name=f"wk_sb{d}") for d in range(NT_D)]
            wv_sb = [wp.tile([128, D], BF16, name=f"wv_sb{d}") for d in range(NT_D)]
            for d in range(NT_D):
                nc.sync.dma_start(wq_sb[d][:], wq[d * 128:(d + 1) * 128, :])
                nc.sync.dma_start(wk_sb[d][:], wk[d * 128:(d + 1) * 128, :])
                nc.sync.dma_start(wv_sb[d][:], wv[d * 128:(d + 1) * 128, :])

            ps = ctx.enter_context(tc.tile_pool(name="b_ps", bufs=3, space="PSUM"))
            scr = ctx.enter_context(tc.tile_pool(name="b_scr", bufs=3))
            st_p = ctx.enter_context(tc.tile_pool(name="b_stats", bufs=6))
            natp = ctx.enter_context(tc.tile_pool(name="b_nat", bufs=3))
            ktp = ctx.enter_context(tc.tile_pool(name="b_ktp", bufs=4))

            def proj(t, w_sb):
                p = ps.tile([128, D], F32, name="p_proj")
                for d in range(NT_D):
                    lhsT = xT[d][:, t * 128:(t + 1) * 128]
                    for ofs, n in _chunks(D):
                        nc.tensor.matmul(
                            p[:, ofs:ofs + n], lhsT, w_sb[d][:, ofs:ofs + n],
                            start=(d == 0), stop=(d == NT_D - 1))
                return p

            def qknorm(p, dst_dram, t):
                sq = scr.tile([128, D], F32, name="sq_b")
                nc.scalar.activation(sq[:], p[:], AF.Square)
                ss = st_p.tile([128, H], F32, name="ss_b")
                nc.vector.tensor_reduce(
                    ss[:], sq[:].rearrange("p (h k) -> p h k", h=H),
                    axis=mybir.AxisListType.X, op=OP.add)
                srt = st_p.tile([128, H], F32, name="srt_b")
                nc.scalar.activation(srt[:], ss[:], AF.Sqrt, bias=eps_t[:], scale=1.0 / HD)
                rs = st_p.tile([128, H], F32, name="rs_b")
                nc.vector.reciprocal(rs[:], srt[:])
                nat = natp.tile([128, D], BF16, name="nat_b")
                rs_view = rs[:].rearrange("p (h o) -> p h o", o=1).broadcast_to([128, H, HD])
                nc.vector.tensor_tensor(
                    out=nat[:].rearrange("p (h k) -> p h k", h=H),
                    in0=p[:].rearrange("p (h k) -> p h k", h=H),
                    in1=rs_view, op=OP.mult)
                nc.gpsimd.dma_start(dst_dram[t * 128:(t + 1) * 128, :], nat[:])

            for t in range(NT_S):
                pk = proj(t, wk_sb)
                qknorm(pk, kh_d, t)
                pv = proj(t, wv_sb)
                nc.vector.tensor_copy(
                    vview[:, t, :, 0:HD],
                    pv[:].rearrange("p (h k) -> p h k", h=H))
                if t < NT_Q:
                    pq = proj(t, wq_sb)
                    qknorm(pq, qh_d, t)
                if (t + 1) % 4 == 0:
                    # transpose the finished 512-row chunk so the B->C
                    # barrier shrinks to just the last chunk's transposes.
                    # DMA-transpose sources must be >=128 cols (xbar tile) —
                    # narrower falls back to a descriptor-explosion path —
                    # so transpose the 128-col head pair, then DVE-split the
                    # halves into the zero-padded per-head KTz tiles.
                    c = t // 4
                    r0, r1 = c * 512, (c + 1) * 512
                    # split-copies and kqsc scaling run on GpSimd: on the
                    # strict-FIFO DVE queue they would block the qknorm
                    # pipeline behind the serialized transpose DMAs
                    for d in range(NT_D):
                        ktc = ktp.tile([128, 512], BF16, name="ktc")
                        nc.sync.dma_start_transpose(
                            ktc[:], kh_d[r0:r1, d * 128:(d + 1) * 128])
                        for e in range(2):
                            nc.gpsimd.tensor_copy(
                                KTz[2 * d + e][64 * e:64 * e + 64, r0:r1],
                                ktc[64 * e:64 * e + 64, :])
                    if c < SQ // 512:
                        for d in range(NT_D):
                            nc.sync.dma_start_transpose(
                                QT[d][:, r0:r1],
                                qh_d[r0:r1, d * 128:(d + 1) * 128])
                            nc.gpsimd.tensor_scalar_mul(
                                QT[d][:, r0:r1], QT[d][:, r0:r1], kqsc_t[:])
        p_xT.release()

        # =============== Phase C: attention ===============
        with ExitStack() as ctx:
            psL = ctx.enter_context(tc.tile_pool(name="c_psL", bufs=2, space="PSUM"))
            psO = ctx.enter_context(tc.tile_pool(name="c_psO", bufs=2, space="PSUM"))
            pp = ctx.enter_context(tc.tile_pool(name="c_p", bufs=6))
            oup = ctx.enter_context(tc.tile_pool(name="c_oU", bufs=5))

            def divide_head(h, oU_h):
                # broadcast 1/denom (held at partition 64 of oU_h) across
                # 64 partitions via a full-footprint selector matmul (row 64
                # of sel_t is ones, rest zeros — 128-contraction keeps HAM
                # warm), then multiply.
                dt, base = h // 2, (h % 2) * 64
                b_ps = psL.tile([128, SQ], F32, name="b_ps", tag="l_ps")
                for ofs, n in _chunks(SQ):
                    nc.tensor.matmul(b_ps[:, ofs:ofs + n], sel_t[:],
                                     oU_h[:, ofs:ofs + n],
                                     start=True, stop=True)
                nc.vector.scalar_tensor_tensor(
                    oT[base:base + 64, dt * SQ:(dt + 1) * SQ],
                    b_ps[0:HD, :], 1.0, oU_h[0:HD, :],
                    op0=OP.bypass, op1=OP.mult)

            pending = []
            for hp in range(H // 2):
                dt = hp
                o_ps = [psO.tile([128, SQ], F32, name=f"o_ps{e}", tag="o_ps")
                        for e in range(2)]
                for t in range(NT_S):
                    l_ps = [psL.tile([128, SQ], F32, name=f"l_ps{e}",
                                     tag="l_ps") for e in range(2)]
                    for e in range(2):  # head 2*hp+e, data rows at 64*e
                        lhsT = KTz[2 * hp + e][:, t * 128:(t + 1) * 128]
                        for j in range(NC_Q):
                            nc.tensor.matmul(
                                l_ps[e][:, j * CH:(j + 1) * CH], lhsT,
                                QT[dt][:, j * CH:(j + 1) * CH],
                                start=True, stop=True)
                    p_t = [None, None]
                    for e in range(2):
                        p_t[e] = pp.tile([128, SQ], BF16, name=f"p_t{e}", tag="p_t")
                        nc.scalar.activation(p_t[e][:], l_ps[e][:], AF.Exp)
                    for e in range(2):
                        h = 2 * hp + e
                        vofs = t * H * VW + h * VW
                        for ofs, n in _chunks(SQ):
                            nc.tensor.matmul(
                                o_ps[e][:, ofs:ofs + n],
                                Vaug[:, vofs:vofs + 128],
                                p_t[e][:, ofs:ofs + n],
                                start=(t == 0), stop=(t == NT_S - 1))
                    if t == 4:
                        for h_prev, oU_prev in pending:
                            divide_head(h_prev, oU_prev)
                        pending = []
                # both PSUM evacuations FIRST (they gate the next head
                # pair's o_ps buffers), then the slow 1-lane reciprocals
                # (6.4us each on DVE's iterative divide — they gate only the
                # deferred divide_head, which is off the critical path).
                oUs = []
                for e in range(2):
                    oU_h = oup.tile([128, SQ], BF16, name="oU", tag="oU")
                    nc.vector.tensor_copy(oU_h[:], o_ps[e][:])
                    oUs.append(oU_h)
                for e in range(2):
                    oU_h = oUs[e]
                    with nc.allow_low_precision(reason="softmax denom recip"):
                        nc.vector.reciprocal(oU_h[HD:HD + 1, :],
                                             oU_h[HD:HD + 1, :])
                    pending.append((2 * hp + e, oU_h))
            for h_prev, oU_prev in pending:
                divide_head(h_prev, oU_prev)
        p_att.release()

        # =============== Phase D: out-proj + residual + ln2 ===============
        with ExitStack() as ctx:
            wp = ctx.enter_context(tc.tile_pool(name="d_w", bufs=1))
            wo_sb = [wp.tile([128, D], BF16, name=f"wo_sb{d}") for d in range(NT_D)]
            for d in range(NT_D):
                nc.sync.dma_start(wo_sb[d][:], wo[d * 128:(d + 1) * 128, :])
            ps = ctx.enter_context(tc.tile_pool(name="d_ps", bufs=2, space="PSUM"))
            io = ctx.enter_context(tc.tile_pool(name="d_io", bufs=3))
            scr = ctx.enter_context(tc.tile_pool(name="d_scr", bufs=3))
            st_p = ctx.enter_context(tc.tile_pool(name="d_stats", bufs=4))

            for q in range(NT_Q):
                p = ps.tile([128, D], F32, name="p_oproj")
                for d in range(NT_D):
                    for ofs, n in _chunks(D):
                        nc.tensor.matmul(
                            p[:, ofs:ofs + n],
                            oT[:, d * SQ + q * 128: d * SQ + (q + 1) * 128],
                            wo_sb[d][:, ofs:ofs + n],
                            start=(d == 0), stop=(d == NT_D - 1))
                lt = io.tile([128, D], F32, name="lt_d")
                nc.sync.dma_start(lt[:], lat[q * 128:(q + 1) * 128, :])
                nc.vector.tensor_tensor(out=x2[q][:], in0=p[:], in1=lt[:], op=OP.add)
                sq = scr.tile([128, D], F32, name="sq_d")
                ssq = st_p.tile([128, 1], F32, name="ssq_d")
                nc.scalar.activation(sq[:], x2[q][:], AF.Square, accum_out=ssq[:])
                srt = st_p.tile([128, 1], F32, name="srt_d")
                nc.scalar.activation(srt[:], ssq[:], AF.Sqrt, bias=eps_t[:], scale=1.0 / D)
                rs = st_p.tile([128, 1], F32, name="rs_d")
                nc.vector.reciprocal(rs[:], srt[:])
                xh2 = scr.tile([128, D], BF16, name="xh2")
                nc.vector.tensor_scalar_mul(xh2[:], x2[q][:], rs[:])
                nc.gpsimd.dma_start(x2h_d[q * 128:(q + 1) * 128, :], xh2[:])
                if (q + 1) % 4 == 0:
                    c = q // 4
                    r0, r1 = c * 512, (c + 1) * 512
                    for d in range(NT_D):
                        nc.sync.dma_start_transpose(
                            x2T[d][:, r0:r1], x2h_d[r0:r1, d * 128:(d + 1) * 128])
        p_oT.release()

        # =============== Phase E: MLP ===============
        p_hT = tc.alloc_tile_pool(name="p_hT", bufs=1)
        hT = ptile(p_hT, [128, NT_M * SQ], BF16, name="hT")
        with ExitStack() as ctx:
            wp = ctx.enter_context(tc.tile_pool(name="e_w", bufs=1))
            wi_sb = [wp.tile([128, MLP], BF16, name=f"wi_sb{d}") for d in range(NT_D)]
            for d in range(NT_D):
                nc.sync.dma_start(wi_sb[d][:], wi[d * 128:(d + 1) * 128, :])
            wom_sb = [wp.tile([128, D], BF16, name=f"wom_sb{m}") for m in range(NT_M)]
            for m in range(NT_M):
                nc.sync.dma_start(wom_sb[m][:], wom[m * 128:(m + 1) * 128, :])

            ps = ctx.enter_context(tc.tile_pool(name="e_ps", bufs=1, space="PSUM"))
            iop = ctx.enter_context(tc.tile_pool(name="e_io", bufs=3))

            # j-outer MLP1: chunk j only needs x2T[:, j*CH:...] so it can
            # start right after phase D's first 4 q-tiles; MLP2 for chunk j
            # then overlaps MLP1 of chunk j+1.
            for j in range(NC_Q):
                for m in range(NT_M):
                    p = ps.tile([128, CH], F32, name="p_mlp1", bufs=2)
                    for d in range(NT_D):
                        nc.tensor.matmul(
                            p[:],
                            wi_sb[d][:, m * 128:(m + 1) * 128],
                            x2T[d][:, j * CH:(j + 1) * CH],
                            start=(d == 0), stop=(d == NT_D - 1))
                    hslc = hT[:, m * SQ + j * CH: m * SQ + (j + 1) * CH]
                    if not sim_compat:
                        nc.scalar.activation(hslc, p[:], AF.Gelu_apprx_tanh)
                    else:
                        xsq = iop.tile([128, CH], F32, name="g_xsq", bufs=1)
                        nc.vector.tensor_tensor(out=xsq[:], in0=p[:], in1=p[:], op=OP.mult)
                        w = iop.tile([128, CH], F32, name="g_w", bufs=1)
                        nc.vector.tensor_scalar(w[:], xsq[:], 0.044715, 1.0,
                                                op0=OP.mult, op1=OP.add)
                        u = iop.tile([128, CH], F32, name="g_u", bufs=1)
                        nc.vector.tensor_tensor(out=u[:], in0=w[:], in1=p[:], op=OP.mult)
                        th = iop.tile([128, CH], F32, name="g_th", bufs=1)
                        nc.scalar.activation(th[:], u[:], AF.Tanh, scale=0.7978845608028654)
                        t2 = iop.tile([128, CH], F32, name="g_t2", bufs=1)
                        nc.vector.scalar_tensor_tensor(t2[:], th[:], 1.0, p[:],
                                                       op0=OP.add, op1=OP.mult)
                        nc.vector.tensor_scalar_mul(hslc, t2[:], 0.5)
                for q in range(j * NT_Q // NC_Q, (j + 1) * NT_Q // NC_Q):
                    p = ps.tile([128, D], F32, name="p_mlp2", bufs=2)
                    for m in range(NT_M):
                        for ofs, n in _chunks(D):
                            nc.tensor.matmul(
                                p[:, ofs:ofs + n],
                                hT[:, m * SQ + q * 128: m * SQ + (q + 1) * 128],
                                wom_sb[m][:, ofs:ofs + n],
                                start=(m == 0), stop=(m == NT_M - 1))
                    ot = iop.tile([128, D], F32, name="ot_e")
                    nc.vector.tensor_tensor(out=ot[:], in0=p[:], in1=x2[q][:], op=OP.add)
                    nc.sync.dma_start(out[q * 128:(q + 1) * 128, :], ot[:])
        p_hT.release()

    nc.compile()
    return nc


def make_in_maps(latents, ln1_scale, wq, wk, wv, q_norm_scale, k_norm_scale,
                 wo_attn, ln2_scale, wi, wo_mlp):
    import ml_dtypes
    bf = ml_dtypes.bfloat16
    wq2 = (np.asarray(ln1_scale, np.float64)[:, None]
           * np.asarray(wq, np.float64).reshape(D, D)).astype(bf)
    wk2 = (np.asarray(ln1_scale, np.float64)[:, None]
           * np.asarray(wk, np.float64).reshape(D, D)).astype(bf)
    wv2 = (np.asarray(ln1_scale, np.float64)[:, None]
           * np.asarray(wv, np.float64).reshape(D, D)).astype(bf)
    wo2 = np.asarray(wo_attn, np.float32).reshape(D, D).astype(bf)
    wi2 = (np.asarray(ln2_scale, np.float64)[:, None]
           * np.asarray(wi, np.float64)).astype(bf)
    wom2 = np.asarray(wo_mlp, np.float32).astype(bf)
    kq = (np.tile(np.asarray(q_norm_scale, np.float64)
                  * np.asarray(k_norm_scale, np.float64), 2)
          / np.sqrt(HD)).astype(np.float32)[:, None]
    lat_np = np.asarray(latents, np.float32)
    ident_np = np.eye(128, dtype=bf)
    in_maps = []
    for c in range(8):
        b, half = c // 2, c % 2
        lm = lat_np[b]
        lat_rot = np.concatenate([lm[half * SQ:(half + 1) * SQ],
                                  lm[(1 - half) * SQ:(2 - half) * SQ]], axis=0)
        in_maps.append(dict(lat=np.ascontiguousarray(lat_rot), wq=wq2, wk=wk2,
                            wv=wv2, wo=wo2, wi=wi2, wom=wom2, kqsc=kq,
                            ident=ident_np))
    return in_maps


_NC_CACHE = None


def kernel(**inputs):
    global _NC_CACHE
    if _NC_CACHE is None:
        _NC_CACHE = build_nc()
    nc = _NC_CACHE
    in_maps = make_in_maps(**inputs)
    res = run_bass_kernel_spmd(nc, in_maps, list(range(8)))
    y = np.empty((B, S, D), np.float32)
    for c in range(8):
        b, half = c // 2, c % 2
        y[b, half * SQ:(half + 1) * SQ] = res.results[c]["out"]
    return y


if __name__ == "__main__":
    import reference
    inputs = {k: np.asarray(v) for k, v in reference.setup_inputs().items()}
    y = kernel(**inputs)
    exp = np.asarray(reference.reference(**reference.setup_inputs()))
    err = np.abs(y - exp).max() / np.abs(exp).max()
    print("Relative error:", err)



# revision 26
# speedup vs baseline: 1.1444x; 1.1444x over previous
"""Fused transformer block (RMSNorm + qk-norm attention + MLP) for TRN2, 8 cores.

Sharding: 8 cores = (4 batches) x (2 query-halves). Each core gets its batch's
full sequence with rows rotated so its query half is rows 0..1023 (attention is
permutation-invariant over keys, so K/V row order doesn't matter). No
collectives needed; each core produces a disjoint [1024, 768] output slice.

Key performance facts this kernel is built around:
  - The PE HAM clock gate only holds 2.4 GHz when the array footprint stays
    high: every attention matmul is framed as a full 128x128-footprint op
    (zero-padded per-head K^T for logits; 128-wide over-read V windows and a
    full-footprint selector broadcast for the softmax divide).
  - ScalarE exp over the 25M logits (~206us) is the attention-phase floor;
    everything else overlaps under it.
  - DVE is strict-FIFO: big memsets go to GpSimd, PSUM evacuations are
    emitted before slow 1-lane reciprocals, and transpose-dependent fixups
    (KTz split copies, Q^T kqsc scaling) run on GpSimd so DMA waits don't
    stall the DVE pipeline.
  - Phase A builds x_hat^T with per-tile PE transposes (no DRAM round trip);
    K/Q/x2 transposes go through DRAM in 512-row chunks (>=128-col sources
    only: narrower DMA-transposes fall back to a descriptor explosion),
    pipelined so the next phase starts before the last chunk lands.
"""

import numpy as np
from contextlib import ExitStack

import concourse.bass as bass
import concourse.tile as tile
from concourse import bacc, mybir
from concourse.bass_utils import run_bass_kernel_spmd

F32 = mybir.dt.float32
BF16 = mybir.dt.bfloat16
AF = mybir.ActivationFunctionType
OP = mybir.AluOpType

B, S, D, H, HD, MLP = 4, 2048, 768, 12, 64, 3072
SQ = S // 2            # query rows per core
NT_S = S // 128        # 16 sequence tiles
NT_Q = SQ // 128       # 8 query tiles
NT_D = D // 128        # 6 model-dim tiles
NT_M = MLP // 128      # 24 mlp-dim tiles
EPS = 1e-6
VW = HD + 1            # V width incl. ones column
CH = 512               # transpose chunk (rows)
NC_S = S // CH         # 4 chunks over full sequence
NC_Q = SQ // CH        # 2 chunks over query rows


def _chunks(n):
    out, ofs = [], 0
    while ofs < n:
        c = min(512, n - ofs)
        out.append((ofs, c))
        ofs += c
    return out


def build_nc(sim_compat=False):
    nc = bacc.Bacc("TRN2", target_bir_lowering=False, debug=False, num_devices=8)

    lat = nc.dram_tensor("lat", [S, D], F32, kind="ExternalInput").ap()
    ident = nc.dram_tensor("ident", [128, 128], BF16, kind="ExternalInput").ap()
    wq = nc.dram_tensor("wq", [D, D], BF16, kind="ExternalInput").ap()
    wk = nc.dram_tensor("wk", [D, D], BF16, kind="ExternalInput").ap()
    wv = nc.dram_tensor("wv", [D, D], BF16, kind="ExternalInput").ap()
    wo = nc.dram_tensor("wo", [D, D], BF16, kind="ExternalInput").ap()
    wi = nc.dram_tensor("wi", [D, MLP], BF16, kind="ExternalInput").ap()
    wom = nc.dram_tensor("wom", [MLP, D], BF16, kind="ExternalInput").ap()
    kqsc = nc.dram_tensor("kqsc", [128, 1], F32, kind="ExternalInput").ap()
    out = nc.dram_tensor("out", [SQ, D], F32, kind="ExternalOutput").ap()

    with tile.TileContext(nc) as tc, ExitStack() as top:
        def ptile(pool, shape, dtype, name):
            return pool.tile(shape, dtype, name=name, tag=name)

        p_const = top.enter_context(tc.tile_pool(name="p_const", bufs=1))
        p_x2 = top.enter_context(tc.tile_pool(name="p_x2", bufs=1))
        p_oT = tc.alloc_tile_pool(name="p_oT", bufs=1)
        p_att = tc.alloc_tile_pool(name="p_att", bufs=1)

        # ---- persistent tiles ----
        # Vaug layout per (t, head): [64 V cols | ones col]; attn@V reads a
        # 128-wide stationary window (full PE-array footprint keeps the HAM
        # clock at 2.4 GHz) that runs into the next head's block (harmless:
        # junk lands in PSUM rows 65-127). The softmax denominator lands in
        # PSUM row 64 so the whole [128,SQ] accumulator is evacuated and
        # re-broadcast with full-footprint aligned ops. 63-col zero tail
        # covers the last head's window.
        Vaug = ptile(p_att, [128, NT_S * H * VW + 63], BF16, name="Vaug")
        oT = ptile(p_oT, [128, NT_D * SQ], BF16, name="oT")
        kqsc_t = ptile(p_const, [128, 1], F32, name="kqsc_t")
        sel_t = ptile(p_const, [128, 128], BF16, name="sel_t")
        eps_t = ptile(p_const, [128, 1], F32, name="eps_t")
        ident_t = ptile(p_const, [128, 128], BF16, name="ident_t")
        nc.sync.dma_start(ident_t[:], ident[:])
        # Per-head K^T with the head's 64 rows at partition 64*(h%2) and
        # zeros in the other 64: logits matmuls contract over the full 128
        # partitions (zero rows contribute nothing) so the PE array runs at
        # 100% footprint instead of 50% — keeps HAM unthrottled.
        KTz = [ptile(p_att, [128, S], BF16, name=f"KTz{h}") for h in range(H)]
        QT = [ptile(p_att, [128, SQ], BF16, name=f"QT{d}") for d in range(NT_D)]
        x2 = [ptile(p_x2, [128, D], BF16, name=f"x2_{q}") for q in range(NT_Q)]
        x2T = [ptile(p_x2, [128, SQ], BF16, name=f"x2T{d}") for d in range(NT_D)]

        nc.sync.dma_start(kqsc_t[:], kqsc[:])
        nc.vector.memset(eps_t[:], EPS)
        # big zero-fills go on the idle GpSimd queue: DVE is strict FIFO and
        # fills here would stall phase A's rmsnorm chain behind them
        nc.gpsimd.memset(sel_t[:], 0.0)
        nc.vector.memset(sel_t[HD:HD + 1, :], 1.0)
        nc.gpsimd.memset(Vaug[:, NT_S * H * VW:], 0.0)
        for h in range(H):
            e = h % 2
            nc.gpsimd.memset(KTz[h][64 * (1 - e):64 * (1 - e) + 64, :], 0.0)
        vview = Vaug[:, 0:NT_S * H * VW].rearrange(
            "p (s h k) -> p s h k", s=NT_S, h=H)
        nc.gpsimd.memset(vview[:, :, :, HD:VW], 1.0)

        dram = top.enter_context(tc.tile_pool(name="dram", bufs=1, space="DRAM"))
        kh_d = dram.tile([S, D], BF16, name="kh_d")
        qh_d = dram.tile([SQ, D], BF16, name="qh_d")
        x2h_d = dram.tile([SQ, D], BF16, name="x2h_d")

        # =============== Phase A: ln1 + x_hat^T ===============
        # x_hat^T is built per-tile with PE transposes (no DRAM round trip,
        # no end-of-phase barrier): projections for tile t can start as soon
        # as tile t is transposed.
        p_xT = tc.alloc_tile_pool(name="p_xT", bufs=1)
        xTa = ptile(p_xT, [128, NT_D * S], BF16, name="xTa")
        xT = [xTa[:, d * S:(d + 1) * S] for d in range(NT_D)]
        with ExitStack() as ctx:
            io = ctx.enter_context(tc.tile_pool(name="a_io", bufs=5))
            st_p = ctx.enter_context(tc.tile_pool(name="a_stats", bufs=8))
            scr = ctx.enter_context(tc.tile_pool(name="a_scr", bufs=5))
            tp_ps = ctx.enter_context(tc.tile_pool(name="a_tps", bufs=4, space="PSUM"))
            for t in range(NT_S):
                lt = io.tile([128, D], F32, name="lt")
                nc.sync.dma_start(lt[:], lat[t * 128:(t + 1) * 128, :])
                sq = scr.tile([128, D], F32, name="sq")
                ssq = st_p.tile([128, 1], F32, name="ssq")
                nc.scalar.activation(sq[:], lt[:], AF.Square, accum_out=ssq[:])
                srt = st_p.tile([128, 1], F32, name="srt")
                nc.scalar.activation(srt[:], ssq[:], AF.Sqrt, bias=eps_t[:], scale=1.0 / D)
                rs = st_p.tile([128, 1], F32, name="rs")
                nc.vector.reciprocal(rs[:], srt[:])
                xh = scr.tile([128, D], BF16, name="xh")
                nc.vector.tensor_scalar_mul(xh[:], lt[:], rs[:])
                tps = tp_ps.tile([128, NT_D * 128], BF16, name="tps")
                for d in range(NT_D):
                    nc.tensor.transpose(tps[:, d * 128:(d + 1) * 128],
                                        xh[:, d * 128:(d + 1) * 128],
                                        ident_t[:])
                nc.vector.tensor_copy(
                    xTa[:].rearrange("p (d s) -> p d s", d=NT_D)[
                        :, :, t * 128:(t + 1) * 128],
                    tps[:].rearrange("p (d s) -> p d s", d=NT_D))

        # =============== Phase B: Q/K/V projections + qk-norm ===============
        with ExitStack() as ctx:
            wp = ctx.enter_context(tc.tile_pool(name="b_w", bufs=1))
            wq_sb = [wp.tile([128, D], BF16, name=f"wq_sb{d}") for d in range(NT_D)]
            wk_sb = [wp.tile([128, D], BF16, name=f"wk_sb{d}") for d in range(NT_D)]
            wv_sb = [wp.tile([128, D], BF16, name=f"wv_sb{d}") for d in range(NT_D)]
            for d in range(NT_D):
                nc.sync.dma_start(wq_sb[d][:], wq[d * 128:(d + 1) * 128, :])
                nc.sync.dma_start(wk_sb[d][:], wk[d * 128:(d + 1) * 128, :])
                nc.sync.dma_start(wv_sb[d][:], wv[d * 128:(d + 1) * 128, :])

            ps = ctx.enter_context(tc.tile_pool(name="b_ps", bufs=3, space="PSUM"))
            scr = ctx.enter_context(tc.tile_pool(name="b_scr", bufs=3))
            st_p = ctx.enter_context(tc.tile_pool(name="b_stats", bufs=6))
            natp = ctx.enter_context(tc.tile_pool(name="b_nat", bufs=3))
            ktp = ctx.enter_context(tc.tile_pool(name="b_ktp", bufs=1))
            ktcs_rot = [[ktp.tile([128, 512], BF16, name=f"ktc{s}_{d}",
                                  tag=f"ktc{s}_{d}")
                         for d in range(NT_D)] for s in range(2)]
            ktcs = [ktcs_rot[c % 2] for c in range(NC_S)]

            def fixup(c):
                r0, r1 = c * 512, (c + 1) * 512
                for d in range(NT_D):
                    for e in range(2):
                        nc.vector.tensor_copy(
                            KTz[2 * d + e][64 * e:64 * e + 64, r0:r1],
                            ktcs[c][d][64 * e:64 * e + 64, :])
                if c < SQ // 512:
                    for d in range(NT_D):
                        nc.vector.tensor_scalar_mul(
                            QT[d][:, r0:r1], QT[d][:, r0:r1], kqsc_t[:])

            def proj(t, w_sb):
                p = ps.tile([128, D], F32, name="p_proj")
                for d in range(NT_D):
                    lhsT = xT[d][:, t * 128:(t + 1) * 128]
                    for ofs, n in _chunks(D):
                        nc.tensor.matmul(
                            p[:, ofs:ofs + n], lhsT, w_sb[d][:, ofs:ofs + n],
                            start=(d == 0), stop=(d == NT_D - 1))
                return p

            def qknorm(p, dst_dram, t):
                sq = scr.tile([128, D], F32, name="sq_b")
                nc.scalar.activation(sq[:], p[:], AF.Square)
                ss = st_p.tile([128, H], F32, name="ss_b")
                nc.vector.tensor_reduce(
                    ss[:], sq[:].rearrange("p (h k) -> p h k", h=H),
                    axis=mybir.AxisListType.X, op=OP.add)
                srt = st_p.tile([128, H], F32, name="srt_b")
                nc.scalar.activation(srt[:], ss[:], AF.Sqrt, bias=eps_t[:], scale=1.0 / HD)
                rs = st_p.tile([128, H], F32, name="rs_b")
                nc.vector.reciprocal(rs[:], srt[:])
                nat = natp.tile([128, D], BF16, name="nat_b")
                rs_view = rs[:].rearrange("p (h o) -> p h o", o=1).broadcast_to([128, H, HD])
                nc.vector.tensor_tensor(
                    out=nat[:].rearrange("p (h k) -> p h k", h=H),
                    in0=p[:].rearrange("p (h k) -> p h k", h=H),
                    in1=rs_view, op=OP.mult)
                nc.gpsimd.dma_start(dst_dram[t * 128:(t + 1) * 128, :], nat[:])

            for t in range(NT_S):
                pk = proj(t, wk_sb)
                qknorm(pk, kh_d, t)
                pv = proj(t, wv_sb)
                nc.vector.tensor_copy(
                    vview[:, t, :, 0:HD],
                    pv[:].rearrange("p (h k) -> p h k", h=H))
                if t < NT_Q:
                    pq = proj(t, wq_sb)
                    qknorm(pq, qh_d, t)
                if (t + 1) % 4 == 0:
                    # transpose the finished 512-row chunk so the B->C
                    # barrier shrinks to just the last chunk's transposes.
                    # DMA-transpose sources must be >=128 cols (xbar tile) —
                    # narrower falls back to a descriptor-explosion path —
                    # so transpose the 128-col head pair, then DVE-split the
                    # halves into the zero-padded per-head KTz tiles.
                    c = t // 4
                    r0, r1 = c * 512, (c + 1) * 512
                    # emit the DVE fixups for the chunk transposed TWO chunks
                    # ago first: its DMAs have long completed, so they don't
                    # stall the strict-FIFO DVE queue behind a DMA wait, and
                    # they free this chunk's rotating ktc slot
                    if c >= 2:
                        fixup(c - 2)
                    for d in range(NT_D):
                        ktc = ktcs[c][d]
                        nc.sync.dma_start_transpose(
                            ktc[:], kh_d[r0:r1, d * 128:(d + 1) * 128])
                    if c < SQ // 512:
                        for d in range(NT_D):
                            nc.sync.dma_start_transpose(
                                QT[d][:, r0:r1],
                                qh_d[r0:r1, d * 128:(d + 1) * 128])
            fixup(NC_S - 2)
            fixup(NC_S - 1)
        p_xT.release()

        # =============== Phase C: attention ===============
        with ExitStack() as ctx:
            psL = ctx.enter_context(tc.tile_pool(name="c_psL", bufs=2, space="PSUM"))
            psO = ctx.enter_context(tc.tile_pool(name="c_psO", bufs=2, space="PSUM"))
            pp = ctx.enter_context(tc.tile_pool(name="c_p", bufs=6))
            oup = ctx.enter_context(tc.tile_pool(name="c_oU", bufs=5))

            def divide_head(h, oU_h):
                # broadcast 1/denom (held at partition 64 of oU_h) across
                # 64 partitions via a full-footprint selector matmul (row 64
                # of sel_t is ones, rest zeros — 128-contraction keeps HAM
                # warm), then multiply.
                dt, base = h // 2, (h % 2) * 64
                b_ps = psL.tile([128, SQ], F32, name="b_ps", tag="l_ps")
                for ofs, n in _chunks(SQ):
                    nc.tensor.matmul(b_ps[:, ofs:ofs + n], sel_t[:],
                                     oU_h[:, ofs:ofs + n],
                                     start=True, stop=True)
                nc.vector.scalar_tensor_tensor(
                    oT[base:base + 64, dt * SQ:(dt + 1) * SQ],
                    b_ps[0:HD, :], 1.0, oU_h[0:HD, :],
                    op0=OP.bypass, op1=OP.mult)

            pending = []
            for hp in range(H // 2):
                dt = hp
                o_ps = [psO.tile([128, SQ], F32, name=f"o_ps{e}", tag="o_ps")
                        for e in range(2)]
                for t in range(NT_S):
                    l_ps = [psL.tile([128, SQ], F32, name=f"l_ps{e}",
                                     tag="l_ps") for e in range(2)]
                    for e in range(2):  # head 2*hp+e, data rows at 64*e
                        lhsT = KTz[2 * hp + e][:, t * 128:(t + 1) * 128]
                        for j in range(NC_Q):
                            nc.tensor.matmul(
                                l_ps[e][:, j * CH:(j + 1) * CH], lhsT,
                                QT[dt][:, j * CH:(j + 1) * CH],
                                start=True, stop=True)
                    p_t = [None, None]
                    for e in range(2):
                        p_t[e] = pp.tile([128, SQ], BF16, name=f"p_t{e}", tag="p_t")
                        nc.scalar.activation(p_t[e][:], l_ps[e][:], AF.Exp)
                    for e in range(2):
                        h = 2 * hp + e
                        vofs = t * H * VW + h * VW
                        for ofs, n in _chunks(SQ):
                            nc.tensor.matmul(
                                o_ps[e][:, ofs:ofs + n],
                                Vaug[:, vofs:vofs + 128],
                                p_t[e][:, ofs:ofs + n],
                                start=(t == 0), stop=(t == NT_S - 1))
                    if t == 4:
                        for h_prev, oU_prev in pending:
                            divide_head(h_prev, oU_prev)
                        pending = []
                # both PSUM evacuations FIRST (they gate the next head
                # pair's o_ps buffers), then the slow 1-lane reciprocals
                # (6.4us each on DVE's iterative divide — they gate only the
                # deferred divide_head, which is off the critical path).
                oUs = []
                for e in range(2):
                    oU_h = oup.tile([128, SQ], BF16, name="oU", tag="oU")
                    nc.vector.tensor_copy(oU_h[:], o_ps[e][:])
                    oUs.append(oU_h)
                for e in range(2):
                    oU_h = oUs[e]
                    with nc.allow_low_precision(reason="softmax denom recip"):
                        nc.vector.reciprocal(oU_h[HD:HD + 1, :],
                                             oU_h[HD:HD + 1, :])
                    pending.append((2 * hp + e, oU_h))
            for h_prev, oU_prev in pending:
                divide_head(h_prev, oU_prev)
        p_att.release()

        # =============== Phase D: out-proj + residual + ln2 ===============
        with ExitStack() as ctx:
            wp = ctx.enter_context(tc.tile_pool(name="d_w", bufs=1))
            wo_sb = [wp.tile([128, D], BF16, name=f"wo_sb{d}") for d in range(NT_D)]
            for d in range(NT_D):
                nc.sync.dma_start(wo_sb[d][:], wo[d * 128:(d + 1) * 128, :])
            ps = ctx.enter_context(tc.tile_pool(name="d_ps", bufs=2, space="PSUM"))
            io = ctx.enter_context(tc.tile_pool(name="d_io", bufs=3))
            scr = ctx.enter_context(tc.tile_pool(name="d_scr", bufs=3))
            st_p = ctx.enter_context(tc.tile_pool(name="d_stats", bufs=4))

            for q in range(NT_Q):
                p = ps.tile([128, D], F32, name="p_oproj")
                for d in range(NT_D):
                    for ofs, n in _chunks(D):
                        nc.tensor.matmul(
                            p[:, ofs:ofs + n],
                            oT[:, d * SQ + q * 128: d * SQ + (q + 1) * 128],
                            wo_sb[d][:, ofs:ofs + n],
                            start=(d == 0), stop=(d == NT_D - 1))
                lt = io.tile([128, D], F32, name="lt_d")
                nc.sync.dma_start(lt[:], lat[q * 128:(q + 1) * 128, :])
                with nc.allow_low_precision(reason="bf16 residual store"):
                    nc.vector.tensor_tensor(out=x2[q][:], in0=p[:], in1=lt[:],
                                            op=OP.add)
                sq = scr.tile([128, D], F32, name="sq_d")
                ssq = st_p.tile([128, 1], F32, name="ssq_d")
                nc.scalar.activation(sq[:], x2[q][:], AF.Square, accum_out=ssq[:])
                srt = st_p.tile([128, 1], F32, name="srt_d")
                nc.scalar.activation(srt[:], ssq[:], AF.Sqrt, bias=eps_t[:], scale=1.0 / D)
                rs = st_p.tile([128, 1], F32, name="rs_d")
                nc.vector.reciprocal(rs[:], srt[:])
                xh2 = scr.tile([128, D], BF16, name="xh2")
                nc.vector.tensor_scalar_mul(xh2[:], x2[q][:], rs[:])
                nc.gpsimd.dma_start(x2h_d[q * 128:(q + 1) * 128, :], xh2[:])
                if (q + 1) % 4 == 0:
                    c = q // 4
                    r0, r1 = c * 512, (c + 1) * 512
                    for d in range(NT_D):
                        nc.sync.dma_start_transpose(
                            x2T[d][:, r0:r1], x2h_d[r0:r1, d * 128:(d + 1) * 128])
        p_oT.release()

        # =============== Phase E: MLP ===============
        p_hT = tc.alloc_tile_pool(name="p_hT", bufs=1)
        hT = ptile(p_hT, [128, NT_M * SQ], BF16, name="hT")
        with ExitStack() as ctx:
            wp = ctx.enter_context(tc.tile_pool(name="e_w", bufs=1))
            wi_sb = [wp.tile([128, MLP], BF16, name=f"wi_sb{d}") for d in range(NT_D)]
            for d in range(NT_D):
                nc.sync.dma_start(wi_sb[d][:], wi[d * 128:(d + 1) * 128, :])
            wom_sb = [wp.tile([128, D], BF16, name=f"wom_sb{m}") for m in range(NT_M)]
            for m in range(NT_M):
                nc.sync.dma_start(wom_sb[m][:], wom[m * 128:(m + 1) * 128, :])

            ps = ctx.enter_context(tc.tile_pool(name="e_ps", bufs=1, space="PSUM"))
            iop = ctx.enter_context(tc.tile_pool(name="e_io", bufs=3))

            # j-outer MLP1: chunk j only needs x2T[:, j*CH:...] so it can
            # start right after phase D's first 4 q-tiles; MLP2 for chunk j
            # then overlaps MLP1 of chunk j+1.
            for j in range(NC_Q):
                for m in range(NT_M):
                    p = ps.tile([128, CH], F32, name="p_mlp1", bufs=2)
                    for d in range(NT_D):
                        nc.tensor.matmul(
                            p[:],
                            wi_sb[d][:, m * 128:(m + 1) * 128],
                            x2T[d][:, j * CH:(j + 1) * CH],
                            start=(d == 0), stop=(d == NT_D - 1))
                    hslc = hT[:, m * SQ + j * CH: m * SQ + (j + 1) * CH]
                    if not sim_compat:
                        nc.scalar.activation(hslc, p[:], AF.Gelu_apprx_tanh)
                    else:
                        xsq = iop.tile([128, CH], F32, name="g_xsq", bufs=1)
                        nc.vector.tensor_tensor(out=xsq[:], in0=p[:], in1=p[:], op=OP.mult)
                        w = iop.tile([128, CH], F32, name="g_w", bufs=1)
                        nc.vector.tensor_scalar(w[:], xsq[:], 0.044715, 1.0,
                                                op0=OP.mult, op1=OP.add)
                        u = iop.tile([128, CH], F32, name="g_u", bufs=1)
                        nc.vector.tensor_tensor(out=u[:], in0=w[:], in1=p[:], op=OP.mult)
                        th = iop.tile([128, CH], F32, name="g_th", bufs=1)
                        nc.scalar.activation(th[:], u[:], AF.Tanh, scale=0.7978845608028654)
                        t2 = iop.tile([128, CH], F32, name="g_t2", bufs=1)
                        nc.vector.scalar_tensor_tensor(t2[:], th[:], 1.0, p[:],
                                                       op0=OP.add, op1=OP.mult)
                        nc.vector.tensor_scalar_mul(hslc, t2[:], 0.5)
                for q in range(j * NT_Q // NC_Q, (j + 1) * NT_Q // NC_Q):
                    p = ps.tile([128, D], F32, name="p_mlp2", bufs=2)
                    for m in range(NT_M):
                        for ofs, n in _chunks(D):
                            nc.tensor.matmul(
                                p[:, ofs:ofs + n],
                                hT[:, m * SQ + q * 128: m * SQ + (q + 1) * 128],
                                wom_sb[m][:, ofs:ofs + n],
                                start=(m == 0), stop=(m == NT_M - 1))
                    ot = iop.tile([128, D], F32, name="ot_e")
                    nc.vector.tensor_tensor(out=ot[:], in0=p[:], in1=x2[q][:], op=OP.add)
                    nc.sync.dma_start(out[q * 128:(q + 1) * 128, :], ot[:])
        p_hT.release()

    nc.compile()
    return nc


def make_in_maps(latents, ln1_scale, wq, wk, wv, q_norm_scale, k_norm_scale,
                 wo_attn, ln2_scale, wi, wo_mlp):
    import ml_dtypes
    bf = ml_dtypes.bfloat16
    wq2 = (np.asarray(ln1_scale, np.float64)[:, None]
           * np.asarray(wq, np.float64).reshape(D, D)).astype(bf)
    wk2 = (np.asarray(ln1_scale, np.float64)[:, None]
           * np.asarray(wk, np.float64).reshape(D, D)).astype(bf)
    wv2 = (np.asarray(ln1_scale, np.float64)[:, None]
           * np.asarray(wv, np.float64).reshape(D, D)).astype(bf)
    wo2 = np.asarray(wo_attn, np.float32).reshape(D, D).astype(bf)
    wi2 = (np.asarray(ln2_scale, np.float64)[:, None]
           * np.asarray(wi, np.float64)).astype(bf)
    wom2 = np.asarray(wo_mlp, np.float32).astype(bf)
    kq = (np.tile(np.asarray(q_norm_scale, np.float64)
                  * np.asarray(k_norm_scale, np.float64), 2)
          / np.sqrt(HD)).astype(np.float32)[:, None]
    lat_np = np.asarray(latents, np.float32)
    ident_np = np.eye(128, dtype=bf)
    in_maps = []
    for c in range(8):
        b, half = c // 2, c % 2
        lm = lat_np[b]
        lat_rot = np.concatenate([lm[half * SQ:(half + 1) * SQ],
                                  lm[(1 - half) * SQ:(2 - half) * SQ]], axis=0)
        in_maps.append(dict(lat=np.ascontiguousarray(lat_rot), wq=wq2, wk=wk2,
                            wv=wv2, wo=wo2, wi=wi2, wom=wom2, kqsc=kq,
                            ident=ident_np))
    return in_maps


_NC_CACHE = None


def kernel(**inputs):
    global _NC_CACHE
    if _NC_CACHE is None:
        _NC_CACHE = build_nc()
    nc = _NC_CACHE
    in_maps = make_in_maps(**inputs)
    res = run_bass_kernel_spmd(nc, in_maps, list(range(8)))
    y = np.empty((B, S, D), np.float32)
    for c in range(8):
        b, half = c // 2, c % 2
        y[b, half * SQ:(half + 1) * SQ] = res.results[c]["out"]
    return y


if __name__ == "__main__":
    import reference
    inputs = {k: np.asarray(v) for k, v in reference.setup_inputs().items()}
    y = kernel(**inputs)
    exp = np.asarray(reference.reference(**reference.setup_inputs()))
    err = np.abs(y - exp).max() / np.abs(exp).max()
    print("Relative error:", err)



# revision 28
# speedup vs baseline: 1.1445x; 1.0001x over previous
"""Fused transformer block (RMSNorm + qk-norm attention + MLP) for TRN2, 8 cores.

Sharding: 8 cores = (4 batches) x (2 query-halves). Each core gets its batch's
full sequence with rows rotated so its query half is rows 0..1023 (attention is
permutation-invariant over keys, so K/V row order doesn't matter). No
collectives needed; each core produces a disjoint [1024, 768] output slice.

Key performance facts this kernel is built around:
  - The PE HAM clock gate only holds 2.4 GHz when the array footprint stays
    high: every attention matmul is framed as a full 128x128-footprint op
    (zero-padded per-head K^T for logits; 128-wide over-read V windows and a
    full-footprint selector broadcast for the softmax divide).
  - ScalarE exp over the 25M logits (~206us) is the attention-phase floor;
    everything else overlaps under it.
  - DVE is strict-FIFO: big memsets go to GpSimd, PSUM evacuations are
    emitted before slow 1-lane reciprocals, and transpose-dependent fixups
    (KTz split copies, Q^T kqsc scaling) run on GpSimd so DMA waits don't
    stall the DVE pipeline.
  - Phase A builds x_hat^T with per-tile PE transposes (no DRAM round trip);
    K/Q/x2 transposes go through DRAM in 512-row chunks (>=128-col sources
    only: narrower DMA-transposes fall back to a descriptor explosion),
    pipelined so the next phase starts before the last chunk lands.
"""

import numpy as np
from contextlib import ExitStack

import concourse.bass as bass
import concourse.tile as tile
from concourse import bacc, mybir
from concourse.bass_utils import run_bass_kernel_spmd

F32 = mybir.dt.float32
BF16 = mybir.dt.bfloat16
AF = mybir.ActivationFunctionType
OP = mybir.AluOpType

B, S, D, H, HD, MLP = 4, 2048, 768, 12, 64, 3072
SQ = S // 2            # query rows per core
NT_S = S // 128        # 16 sequence tiles
NT_Q = SQ // 128       # 8 query tiles
NT_D = D // 128        # 6 model-dim tiles
NT_M = MLP // 128      # 24 mlp-dim tiles
EPS = 1e-6
VW = HD + 1            # V width incl. ones column
CH = 512               # transpose chunk (rows)
NC_S = S // CH         # 4 chunks over full sequence
NC_Q = SQ // CH        # 2 chunks over query rows


def _chunks(n):
    out, ofs = [], 0
    while ofs < n:
        c = min(512, n - ofs)
        out.append((ofs, c))
        ofs += c
    return out


def build_nc(sim_compat=False):
    nc = bacc.Bacc("TRN2", target_bir_lowering=False, debug=False, num_devices=8)

    lat = nc.dram_tensor("lat", [S, D], F32, kind="ExternalInput").ap()
    ident = nc.dram_tensor("ident", [128, 128], BF16, kind="ExternalInput").ap()
    wq = nc.dram_tensor("wq", [D, D], BF16, kind="ExternalInput").ap()
    wk = nc.dram_tensor("wk", [D, D], BF16, kind="ExternalInput").ap()
    wv = nc.dram_tensor("wv", [D, D], BF16, kind="ExternalInput").ap()
    wo = nc.dram_tensor("wo", [D, D], BF16, kind="ExternalInput").ap()
    wi = nc.dram_tensor("wi", [D, MLP], BF16, kind="ExternalInput").ap()
    wom = nc.dram_tensor("wom", [MLP, D], BF16, kind="ExternalInput").ap()
    kqsc = nc.dram_tensor("kqsc", [128, 1], F32, kind="ExternalInput").ap()
    out = nc.dram_tensor("out", [SQ, D], F32, kind="ExternalOutput").ap()

    with tile.TileContext(nc) as tc, ExitStack() as top:
        def ptile(pool, shape, dtype, name):
            return pool.tile(shape, dtype, name=name, tag=name)

        p_const = top.enter_context(tc.tile_pool(name="p_const", bufs=1))
        p_x2 = top.enter_context(tc.tile_pool(name="p_x2", bufs=1))
        p_oT = tc.alloc_tile_pool(name="p_oT", bufs=1)
        p_att = tc.alloc_tile_pool(name="p_att", bufs=1)

        # ---- persistent tiles ----
        # Vaug layout per (t, head): [64 V cols | ones col]; attn@V reads a
        # 128-wide stationary window (full PE-array footprint keeps the HAM
        # clock at 2.4 GHz) that runs into the next head's block (harmless:
        # junk lands in PSUM rows 65-127). The softmax denominator lands in
        # PSUM row 64 so the whole [128,SQ] accumulator is evacuated and
        # re-broadcast with full-footprint aligned ops. 63-col zero tail
        # covers the last head's window.
        VTW = H * VW + 63      # per-t V tile width (63-col zero tail)
        Vt = [ptile(p_att, [128, VTW], BF16, name=f"Vt{t}")
              for t in range(NT_S)]
        oTd = [ptile(p_oT, [128, SQ], BF16, name=f"oTd{d}")
               for d in range(NT_D)]
        kqsc_t = ptile(p_const, [128, 1], F32, name="kqsc_t")
        sel_t = ptile(p_const, [128, 128], BF16, name="sel_t")
        eps_t = ptile(p_const, [128, 1], F32, name="eps_t")
        ident_t = ptile(p_const, [128, 128], BF16, name="ident_t")
        nc.sync.dma_start(ident_t[:], ident[:])
        # Per-head K^T with the head's 64 rows at partition 64*(h%2) and
        # zeros in the other 64: logits matmuls contract over the full 128
        # partitions (zero rows contribute nothing) so the PE array runs at
        # 100% footprint instead of 50% — keeps HAM unthrottled.
        KTz = [ptile(p_att, [128, S], BF16, name=f"KTz{h}") for h in range(H)]
        QT = [ptile(p_att, [128, SQ], BF16, name=f"QT{d}") for d in range(NT_D)]
        x2 = [ptile(p_x2, [128, D], BF16, name=f"x2_{q}") for q in range(NT_Q)]
        x2T = [ptile(p_x2, [128, SQ], BF16, name=f"x2T{d}") for d in range(NT_D)]

        nc.sync.dma_start(kqsc_t[:], kqsc[:])
        nc.vector.memset(eps_t[:], EPS)
        # big zero-fills go on the idle GpSimd queue: DVE is strict FIFO and
        # fills here would stall phase A's rmsnorm chain behind them
        nc.gpsimd.memset(sel_t[:], 0.0)
        nc.vector.memset(sel_t[HD:HD + 1, :], 1.0)
        for t in range(NT_S):
            nc.gpsimd.memset(Vt[t][:, H * VW:], 0.0)
        for h in range(H):
            e = h % 2
            nc.gpsimd.memset(KTz[h][64 * (1 - e):64 * (1 - e) + 64, :], 0.0)
        vview = [Vt[t][:, 0:H * VW].rearrange("p (h k) -> p h k", h=H)
                 for t in range(NT_S)]
        for t in range(NT_S):
            nc.gpsimd.memset(vview[t][:, :, HD:VW], 1.0)

        dram = top.enter_context(tc.tile_pool(name="dram", bufs=1, space="DRAM"))
        kh_d = dram.tile([S, D], BF16, name="kh_d")
        qh_d = dram.tile([SQ, D], BF16, name="qh_d")
        x2h_d = dram.tile([SQ, D], BF16, name="x2h_d")

        # =============== Phase A: ln1 + x_hat^T ===============
        # x_hat^T is built per-tile with PE transposes (no DRAM round trip,
        # no end-of-phase barrier): projections for tile t can start as soon
        # as tile t is transposed.
        p_xT = tc.alloc_tile_pool(name="p_xT", bufs=1)
        xT = [ptile(p_xT, [128, S], BF16, name=f"xT{d}") for d in range(NT_D)]
        with ExitStack() as ctx:
            io = ctx.enter_context(tc.tile_pool(name="a_io", bufs=5))
            st_p = ctx.enter_context(tc.tile_pool(name="a_stats", bufs=8))
            scr = ctx.enter_context(tc.tile_pool(name="a_scr", bufs=5))
            tp_ps = ctx.enter_context(tc.tile_pool(name="a_tps", bufs=4, space="PSUM"))
            for t in range(NT_S):
                lt = io.tile([128, D], F32, name="lt")
                nc.sync.dma_start(lt[:], lat[t * 128:(t + 1) * 128, :])
                sq = scr.tile([128, D], F32, name="sq")
                ssq = st_p.tile([128, 1], F32, name="ssq")
                nc.scalar.activation(sq[:], lt[:], AF.Square, accum_out=ssq[:])
                srt = st_p.tile([128, 1], F32, name="srt")
                nc.scalar.activation(srt[:], ssq[:], AF.Sqrt, bias=eps_t[:], scale=1.0 / D)
                rs = st_p.tile([128, 1], F32, name="rs")
                nc.vector.reciprocal(rs[:], srt[:])
                xh = scr.tile([128, D], BF16, name="xh")
                nc.vector.tensor_scalar_mul(xh[:], lt[:], rs[:])
                tps = tp_ps.tile([128, NT_D * 128], BF16, name="tps")
                for d in range(NT_D):
                    nc.tensor.transpose(tps[:, d * 128:(d + 1) * 128],
                                        xh[:, d * 128:(d + 1) * 128],
                                        ident_t[:])
                for d in range(NT_D):
                    nc.vector.tensor_copy(xT[d][:, t * 128:(t + 1) * 128],
                                          tps[:, d * 128:(d + 1) * 128])

        # =============== Phase B: Q/K/V projections + qk-norm ===============
        with ExitStack() as ctx:
            wp = ctx.enter_context(tc.tile_pool(name="b_w", bufs=1))
            wq_sb = [wp.tile([128, D], BF16, name=f"wq_sb{d}") for d in range(NT_D)]
            wk_sb = [wp.tile([128, D], BF16, name=f"wk_sb{d}") for d in range(NT_D)]
            wv_sb = [wp.tile([128, D], BF16, name=f"wv_sb{d}") for d in range(NT_D)]
            for d in range(NT_D):
                nc.sync.dma_start(wq_sb[d][:], wq[d * 128:(d + 1) * 128, :])
                nc.sync.dma_start(wk_sb[d][:], wk[d * 128:(d + 1) * 128, :])
                nc.sync.dma_start(wv_sb[d][:], wv[d * 128:(d + 1) * 128, :])

            ps = ctx.enter_context(tc.tile_pool(name="b_ps", bufs=3, space="PSUM"))
            scr = ctx.enter_context(tc.tile_pool(name="b_scr", bufs=3))
            st_p = ctx.enter_context(tc.tile_pool(name="b_stats", bufs=6))
            natp = ctx.enter_context(tc.tile_pool(name="b_nat", bufs=3))
            ktp = ctx.enter_context(tc.tile_pool(name="b_ktp", bufs=1))
            ktcs_rot = [[ktp.tile([128, 512], BF16, name=f"ktc{s}_{d}",
                                  tag=f"ktc{s}_{d}")
                         for d in range(NT_D)] for s in range(2)]
            ktcs = [ktcs_rot[c % 2] for c in range(NC_S)]

            def fixup(c):
                r0, r1 = c * 512, (c + 1) * 512
                for d in range(NT_D):
                    for e in range(2):
                        nc.vector.tensor_copy(
                            KTz[2 * d + e][64 * e:64 * e + 64, r0:r1],
                            ktcs[c][d][64 * e:64 * e + 64, :])
                if c < SQ // 512:
                    for d in range(NT_D):
                        nc.vector.tensor_scalar_mul(
                            QT[d][:, r0:r1], QT[d][:, r0:r1], kqsc_t[:])

            def proj(t, w_sb):
                p = ps.tile([128, D], F32, name="p_proj")
                for d in range(NT_D):
                    lhsT = xT[d][:, t * 128:(t + 1) * 128]
                    for ofs, n in _chunks(D):
                        nc.tensor.matmul(
                            p[:, ofs:ofs + n], lhsT, w_sb[d][:, ofs:ofs + n],
                            start=(d == 0), stop=(d == NT_D - 1))
                return p

            def qknorm(p, dst_dram, t):
                sq = scr.tile([128, D], F32, name="sq_b")
                nc.scalar.activation(sq[:], p[:], AF.Square)
                ss = st_p.tile([128, H], F32, name="ss_b")
                nc.vector.tensor_reduce(
                    ss[:], sq[:].rearrange("p (h k) -> p h k", h=H),
                    axis=mybir.AxisListType.X, op=OP.add)
                srt = st_p.tile([128, H], F32, name="srt_b")
                nc.scalar.activation(srt[:], ss[:], AF.Sqrt, bias=eps_t[:], scale=1.0 / HD)
                rs = st_p.tile([128, H], F32, name="rs_b")
                nc.vector.reciprocal(rs[:], srt[:])
                nat = natp.tile([128, D], BF16, name="nat_b")
                rs_view = rs[:].rearrange("p (h o) -> p h o", o=1).broadcast_to([128, H, HD])
                nc.vector.tensor_tensor(
                    out=nat[:].rearrange("p (h k) -> p h k", h=H),
                    in0=p[:].rearrange("p (h k) -> p h k", h=H),
                    in1=rs_view, op=OP.mult)
                nc.gpsimd.dma_start(dst_dram[t * 128:(t + 1) * 128, :], nat[:])

            for t in range(NT_S):
                pk = proj(t, wk_sb)
                qknorm(pk, kh_d, t)
                pv = proj(t, wv_sb)
                nc.vector.tensor_copy(
                    vview[t][:, :, 0:HD],
                    pv[:].rearrange("p (h k) -> p h k", h=H))
                if t < NT_Q:
                    pq = proj(t, wq_sb)
                    qknorm(pq, qh_d, t)
                if (t + 1) % 4 == 0:
                    # transpose the finished 512-row chunk so the B->C
                    # barrier shrinks to just the last chunk's transposes.
                    # DMA-transpose sources must be >=128 cols (xbar tile) —
                    # narrower falls back to a descriptor-explosion path —
                    # so transpose the 128-col head pair, then DVE-split the
                    # halves into the zero-padded per-head KTz tiles.
                    c = t // 4
                    r0, r1 = c * 512, (c + 1) * 512
                    # emit the DVE fixups for the chunk transposed TWO chunks
                    # ago first: its DMAs have long completed, so they don't
                    # stall the strict-FIFO DVE queue behind a DMA wait, and
                    # they free this chunk's rotating ktc slot
                    if c >= 2:
                        fixup(c - 2)
                    for d in range(NT_D):
                        ktc = ktcs[c][d]
                        nc.sync.dma_start_transpose(
                            ktc[:], kh_d[r0:r1, d * 128:(d + 1) * 128])
                    if c < SQ // 512:
                        for d in range(NT_D):
                            nc.sync.dma_start_transpose(
                                QT[d][:, r0:r1],
                                qh_d[r0:r1, d * 128:(d + 1) * 128])
            fixup(NC_S - 2)
            fixup(NC_S - 1)
        p_xT.release()

        # =============== Phase C: attention ===============
        with ExitStack() as ctx:
            psL = ctx.enter_context(tc.tile_pool(name="c_psL", bufs=2, space="PSUM"))
            psO = ctx.enter_context(tc.tile_pool(name="c_psO", bufs=2, space="PSUM"))
            pp = ctx.enter_context(tc.tile_pool(name="c_p", bufs=6))
            oup = ctx.enter_context(tc.tile_pool(name="c_oU", bufs=5))

            def divide_head(h, oU_h):
                # broadcast 1/denom (held at partition 64 of oU_h) across
                # 64 partitions via a full-footprint selector matmul (row 64
                # of sel_t is ones, rest zeros — 128-contraction keeps HAM
                # warm), then multiply.
                dt, base = h // 2, (h % 2) * 64
                b_ps = psL.tile([128, SQ], F32, name="b_ps", tag="l_ps")
                for ofs, n in _chunks(SQ):
                    nc.tensor.matmul(b_ps[:, ofs:ofs + n], sel_t[:],
                                     oU_h[:, ofs:ofs + n],
                                     start=True, stop=True)
                nc.vector.scalar_tensor_tensor(
                    oTd[dt][base:base + 64, :],
                    b_ps[0:HD, :], 1.0, oU_h[0:HD, :],
                    op0=OP.bypass, op1=OP.mult)

            pending = []
            for hp in range(H // 2):
                dt = hp
                o_ps = [psO.tile([128, SQ], F32, name=f"o_ps{e}", tag="o_ps")
                        for e in range(2)]
                for t in range(NT_S):
                    l_ps = [psL.tile([128, SQ], F32, name=f"l_ps{e}",
                                     tag="l_ps") for e in range(2)]
                    for e in range(2):  # head 2*hp+e, data rows at 64*e
                        lhsT = KTz[2 * hp + e][:, t * 128:(t + 1) * 128]
                        for j in range(NC_Q):
                            nc.tensor.matmul(
                                l_ps[e][:, j * CH:(j + 1) * CH], lhsT,
                                QT[dt][:, j * CH:(j + 1) * CH],
                                start=True, stop=True)
                    p_t = [None, None]
                    for e in range(2):
                        p_t[e] = pp.tile([128, SQ], BF16, name=f"p_t{e}", tag="p_t")
                        nc.scalar.activation(p_t[e][:], l_ps[e][:], AF.Exp)
                    for e in range(2):
                        h = 2 * hp + e
                        for ofs, n in _chunks(SQ):
                            nc.tensor.matmul(
                                o_ps[e][:, ofs:ofs + n],
                                Vt[t][:, h * VW:h * VW + 128],
                                p_t[e][:, ofs:ofs + n],
                                start=(t == 0), stop=(t == NT_S - 1))
                    if t == 4:
                        for h_prev, oU_prev in pending:
                            divide_head(h_prev, oU_prev)
                        pending = []
                # both PSUM evacuations FIRST (they gate the next head
                # pair's o_ps buffers), then the slow 1-lane reciprocals
                # (6.4us each on DVE's iterative divide — they gate only the
                # deferred divide_head, which is off the critical path).
                oUs = []
                for e in range(2):
                    oU_h = oup.tile([128, SQ], BF16, name="oU", tag="oU")
                    nc.vector.tensor_copy(oU_h[:], o_ps[e][:])
                    oUs.append(oU_h)
                for e in range(2):
                    oU_h = oUs[e]
                    with nc.allow_low_precision(reason="softmax denom recip"):
                        nc.vector.reciprocal(oU_h[HD:HD + 1, :],
                                             oU_h[HD:HD + 1, :])
                    pending.append((2 * hp + e, oU_h))
            for h_prev, oU_prev in pending:
                divide_head(h_prev, oU_prev)
        p_att.release()

        # =============== Phase D: out-proj + residual + ln2 ===============
        with ExitStack() as ctx:
            wp = ctx.enter_context(tc.tile_pool(name="d_w", bufs=1))
            wo_sb = [wp.tile([128, D], BF16, name=f"wo_sb{d}") for d in range(NT_D)]
            for d in range(NT_D):
                nc.sync.dma_start(wo_sb[d][:], wo[d * 128:(d + 1) * 128, :])
            ps = ctx.enter_context(tc.tile_pool(name="d_ps", bufs=2, space="PSUM"))
            io = ctx.enter_context(tc.tile_pool(name="d_io", bufs=3))
            scr = ctx.enter_context(tc.tile_pool(name="d_scr", bufs=3))
            st_p = ctx.enter_context(tc.tile_pool(name="d_stats", bufs=4))

            for q in range(NT_Q):
                p = ps.tile([128, D], F32, name="p_oproj")
                for d in range(NT_D):
                    for ofs, n in _chunks(D):
                        nc.tensor.matmul(
                            p[:, ofs:ofs + n],
                            oTd[d][:, q * 128:(q + 1) * 128],
                            wo_sb[d][:, ofs:ofs + n],
                            start=(d == 0), stop=(d == NT_D - 1))
                lt = io.tile([128, D], F32, name="lt_d")
                nc.sync.dma_start(lt[:], lat[q * 128:(q + 1) * 128, :])
                with nc.allow_low_precision(reason="bf16 residual store"):
                    nc.vector.tensor_tensor(out=x2[q][:], in0=p[:], in1=lt[:],
                                            op=OP.add)
                sq = scr.tile([128, D], F32, name="sq_d")
                ssq = st_p.tile([128, 1], F32, name="ssq_d")
                nc.scalar.activation(sq[:], x2[q][:], AF.Square, accum_out=ssq[:])
                srt = st_p.tile([128, 1], F32, name="srt_d")
                nc.scalar.activation(srt[:], ssq[:], AF.Sqrt, bias=eps_t[:], scale=1.0 / D)
                rs = st_p.tile([128, 1], F32, name="rs_d")
                nc.vector.reciprocal(rs[:], srt[:])
                xh2 = scr.tile([128, D], BF16, name="xh2")
                nc.vector.tensor_scalar_mul(xh2[:], x2[q][:], rs[:])
                nc.gpsimd.dma_start(x2h_d[q * 128:(q + 1) * 128, :], xh2[:])
                if (q + 1) % 4 == 0:
                    c = q // 4
                    r0, r1 = c * 512, (c + 1) * 512
                    for d in range(NT_D):
                        nc.sync.dma_start_transpose(
                            x2T[d][:, r0:r1], x2h_d[r0:r1, d * 128:(d + 1) * 128])
        p_oT.release()

        # =============== Phase E: MLP ===============
        p_hT = tc.alloc_tile_pool(name="p_hT", bufs=1)
        hTm = [ptile(p_hT, [128, SQ], BF16, name=f"hTm{m}")
               for m in range(NT_M)]
        with ExitStack() as ctx:
            wp = ctx.enter_context(tc.tile_pool(name="e_w", bufs=1))
            wi_sb = [wp.tile([128, MLP], BF16, name=f"wi_sb{d}") for d in range(NT_D)]
            for d in range(NT_D):
                nc.sync.dma_start(wi_sb[d][:], wi[d * 128:(d + 1) * 128, :])
            wom_sb = [wp.tile([128, D], BF16, name=f"wom_sb{m}") for m in range(NT_M)]
            for m in range(NT_M):
                nc.sync.dma_start(wom_sb[m][:], wom[m * 128:(m + 1) * 128, :])

            ps = ctx.enter_context(tc.tile_pool(name="e_ps", bufs=1, space="PSUM"))
            iop = ctx.enter_context(tc.tile_pool(name="e_io", bufs=3))

            # j-outer MLP1: chunk j only needs x2T[:, j*CH:...] so it can
            # start right after phase D's first 4 q-tiles; MLP2 for chunk j
            # then overlaps MLP1 of chunk j+1.
            for j in range(NC_Q):
                for m in range(NT_M):
                    p = ps.tile([128, CH], F32, name="p_mlp1", bufs=2)
                    for d in range(NT_D):
                        nc.tensor.matmul(
                            p[:],
                            wi_sb[d][:, m * 128:(m + 1) * 128],
                            x2T[d][:, j * CH:(j + 1) * CH],
                            start=(d == 0), stop=(d == NT_D - 1))
                    hslc = hTm[m][:, j * CH:(j + 1) * CH]
                    if not sim_compat:
                        nc.scalar.activation(hslc, p[:], AF.Gelu_apprx_tanh)
                    else:
                        xsq = iop.tile([128, CH], F32, name="g_xsq", bufs=1)
                        nc.vector.tensor_tensor(out=xsq[:], in0=p[:], in1=p[:], op=OP.mult)
                        w = iop.tile([128, CH], F32, name="g_w", bufs=1)
                        nc.vector.tensor_scalar(w[:], xsq[:], 0.044715, 1.0,
                                                op0=OP.mult, op1=OP.add)
                        u = iop.tile([128, CH], F32, name="g_u", bufs=1)
                        nc.vector.tensor_tensor(out=u[:], in0=w[:], in1=p[:], op=OP.mult)
                        th = iop.tile([128, CH], F32, name="g_th", bufs=1)
                        nc.scalar.activation(th[:], u[:], AF.Tanh, scale=0.7978845608028654)
                        t2 = iop.tile([128, CH], F32, name="g_t2", bufs=1)
                        nc.vector.scalar_tensor_tensor(t2[:], th[:], 1.0, p[:],
                                                       op0=OP.add, op1=OP.mult)
                        nc.vector.tensor_scalar_mul(hslc, t2[:], 0.5)
                for q in range(j * NT_Q // NC_Q, (j + 1) * NT_Q // NC_Q):
                    p = ps.tile([128, D], F32, name="p_mlp2", bufs=2)
                    for m in range(NT_M):
                        for ofs, n in _chunks(D):
                            nc.tensor.matmul(
                                p[:, ofs:ofs + n],
                                hTm[m][:, q * 128:(q + 1) * 128],
                                wom_sb[m][:, ofs:ofs + n],
                                start=(m == 0), stop=(m == NT_M - 1))
                    ot = iop.tile([128, D], F32, name="ot_e")
                    nc.vector.tensor_tensor(out=ot[:], in0=p[:], in1=x2[q][:], op=OP.add)
                    nc.sync.dma_start(out[q * 128:(q + 1) * 128, :], ot[:])
        p_hT.release()

    nc.compile()
    return nc


def make_in_maps(latents, ln1_scale, wq, wk, wv, q_norm_scale, k_norm_scale,
                 wo_attn, ln2_scale, wi, wo_mlp):
    import ml_dtypes
    bf = ml_dtypes.bfloat16
    wq2 = (np.asarray(ln1_scale, np.float64)[:, None]
           * np.asarray(wq, np.float64).reshape(D, D)).astype(bf)
    wk2 = (np.asarray(ln1_scale, np.float64)[:, None]
           * np.asarray(wk, np.float64).reshape(D, D)).astype(bf)
    wv2 = (np.asarray(ln1_scale, np.float64)[:, None]
           * np.asarray(wv, np.float64).reshape(D, D)).astype(bf)
    wo2 = np.asarray(wo_attn, np.float32).reshape(D, D).astype(bf)
    wi2 = (np.asarray(ln2_scale, np.float64)[:, None]
           * np.asarray(wi, np.float64)).astype(bf)
    wom2 = np.asarray(wo_mlp, np.float32).astype(bf)
    kq = (np.tile(np.asarray(q_norm_scale, np.float64)
                  * np.asarray(k_norm_scale, np.float64), 2)
          / np.sqrt(HD)).astype(np.float32)[:, None]
    lat_np = np.asarray(latents, np.float32)
    ident_np = np.eye(128, dtype=bf)
    in_maps = []
    for c in range(8):
        b, half = c // 2, c % 2
        lm = lat_np[b]
        lat_rot = np.concatenate([lm[half * SQ:(half + 1) * SQ],
                                  lm[(1 - half) * SQ:(2 - half) * SQ]], axis=0)
        in_maps.append(dict(lat=np.ascontiguousarray(lat_rot), wq=wq2, wk=wk2,
                            wv=wv2, wo=wo2, wi=wi2, wom=wom2, kqsc=kq,
                            ident=ident_np))
    return in_maps


_NC_CACHE = None


def kernel(**inputs):
    global _NC_CACHE
    if _NC_CACHE is None:
        _NC_CACHE = build_nc()
    nc = _NC_CACHE
    in_maps = make_in_maps(**inputs)
    res = run_bass_kernel_spmd(nc, in_maps, list(range(8)))
    y = np.empty((B, S, D), np.float32)
    for c in range(8):
        b, half = c // 2, c % 2
        y[b, half * SQ:(half + 1) * SQ] = res.results[c]["out"]
    return y


if __name__ == "__main__":
    import reference
    inputs = {k: np.asarray(v) for k, v in reference.setup_inputs().items()}
    y = kernel(**inputs)
    exp = np.asarray(reference.reference(**reference.setup_inputs()))
    err = np.abs(y - exp).max() / np.abs(exp).max()
    print("Relative error:", err)

